# revision 37
# baseline (speedup 1.0000x reference)
"""Trainium2 Bass kernel for nn_ClusterMemory (scatter_memory).

Math: the reference loss reduces exactly to
    x_mean[64,256]  = mean over K=8 of L2-normalized input rows      (B=512=16p*4c*8k)
    dots[64,N]      = x_mean @ features.T          (proxies = dots * 20)
    per (p,c) row:  sumexp = sum_n exp(20*dots),  min_exp = min_n exp(20*dots),
                    own[p,c] = 20 * dots[p*4+c, p]
    pos[p] = own[p, argmin_c own[p,:]]
    loss   = mean_p( log(exp(pos) + sum_c(sumexp - min_exp)) - pos )
(log-softmax is permutation invariant; the argsort in the reference only
drops the per-(p,c) minimum from the negative set, and exp is monotone so
the dropped term is min_exp.)

Distribution: features [65536, 256] is sharded over N across 8 cores
(8192 rows each, tensor/column parallel per the sharding hint); the
[512,256] inputs are replicated.  Each core reduces its N-shard to a
[64]-vector of sumexp and min_exp on device; the host adds 8 tiny
partials and finishes the scalar logsumexp in f64 (own/pos is B-sized,
computed on host in f64).

Per-core device pipeline (memory-bound design, ~8 MB of HBM reads):
  1. prologue: normalize x rows, fold the K-mean into a gpsimd-built
     averaging matrix, PE matmul -> xmT [256, 64] cast to bf16.
  2. per 2048-row feature chunk:
     - SWDGE (gpsimd) DMA with f32->bf16 cast: two natural-layout tiles
       natA/natB [128n, 16j*128d] (d-halves 0:128 / 128:256).
     - one SBUF->SBUF DMA-transpose each (xbar, 2-byte) -> fT [128d, 2048n].
     - PE: bf16 matmuls into PSUM [64, 512] (f32 accumulate).
     - ACT: exp(scale=20) with fused row-sum; DVE: row-min of exp.
The bf16 cast costs ~1e-4..1e-3 relative on the final scalar (PSUM
accumulation stays f32; own/pos exact on host), far inside tolerance.
"""

import numpy as np

import concourse.bass as bass
import concourse.tile as tile
from concourse import bacc, mybir
from concourse.bass_utils import run_bass_kernel_spmd

P, C, K = 16, 4, 8
B = P * C * K          # 512
N, D = 65536, 256
PC = P * C             # 64 proxy rows
NCORES = 8
NSHARD = N // NCORES   # 8192
SCALE = 20.0           # 1 / TEMP(0.05)

J = 8                  # 128-row groups per chunk
CHUNK = 128 * J        # 1024 feature rows per chunk
NCHUNK = NSHARD // CHUNK   # 8
MMN = 512              # matmul free dim / PSUM bank
F32 = mybir.dt.float32
BF16 = mybir.dt.bfloat16


def _build_module():
    nc = bacc.Bacc(
        "TRN2",
        target_bir_lowering=False,
        debug=False,
        enable_asserts=False,
        num_devices=NCORES,
    )

    x_in = nc.dram_tensor("x", [B, D], F32, kind="ExternalInput")
    f_in = nc.dram_tensor("feat", [NSHARD, D], F32, kind="ExternalInput")
    se_out = nc.dram_tensor("sumexp", [PC, 1], F32, kind="ExternalOutput")

    with tile.TileContext(nc) as tc:
        with (
            tc.tile_pool(name="singles", bufs=1) as singles,
            tc.tile_pool(name="nat", bufs=3) as nat_pool,
            tc.tile_pool(name="ftT", bufs=3) as ftT_pool,
            tc.tile_pool(name="expst", bufs=2) as exp_pool,
            tc.tile_pool(name="psum_mm", bufs=4, space="PSUM") as psum_mm_pool,
            tc.tile_pool(name="psum_pre", bufs=1, space="PSUM") as psum_pre_pool,
        ):
            # Load layout: each partition holds Q=4 *consecutive* DRAM rows so
            # DMA packets are 4KB contiguous (1KB rows alone run the SDMA
            # engines at ~69% efficiency).  Row n = c*CHUNK + g*512 + p*4 + q;
            # the resulting n-permutation is harmless (sumexp is reduced over
            # all n).
            QR = 4
            G = CHUNK // (128 * QR)   # 4 row-groups per chunk
            f_v = f_in.rearrange("(c g p q) d -> c p g q d", p=128, q=QR, g=G)
            se_cols = singles.tile([PC, NSHARD // MMN], F32, tag="se_cols")

            def frontend(c):
                """Load chunk c (f32, 4KB packets), cast to bf16 on DVE,
                xbar-transpose both d-halves SBUF->SBUF (one per HWDGE ring)."""
                nat32 = nat_pool.tile([128, G, QR, D], F32, tag="nat32", name="nat32")
                nc.sync.dma_start(out=nat32, in_=f_v[c])
                fT = [None, None]
                for dc in range(2):
                    nat = nat_pool.tile(
                        [128, G * QR, 128], BF16, tag=f"nat{dc}", name=f"nat{dc}"
                    )
                    nc.vector.tensor_copy(
                        out=nat.rearrange("p (g q) dd -> p g q dd", g=G),
                        in_=nat32[:, :, :, dc * 128 : (dc + 1) * 128],
                    )
                    fT[dc] = ftT_pool.tile(
                        [128, G * QR, 128], BF16, tag=f"fT{dc}", name=f"fT{dc}"
                    )
                    # xbar transpose SBUF->SBUF: fT[d, j, p] = nat[p, j*128+d].
                    # The ~2.4us trigger blocks the issuing sequencer, so the
                    # two d-halves go to the two HWDGE rings (SP and ACT) and
                    # the loads on SP aren't chained behind both.
                    eng = nc.sync if dc == 0 else nc.scalar
                    eng.dma_start(out=fT[dc], in_=nat, transpose=True)
                return fT

            # ---------------- prologue: xmT = xn.T @ A.T ----------------
            # avg_s[p, bc, g] = 1/8 iff g == (bc*128 + p) // 8  (row b = bc*128+p)
            avg_s = singles.tile([128, 4, PC], F32, tag="avg_s")
            nc.gpsimd.memset(avg_s, 1.0 / K)
            nc.gpsimd.affine_select(
                out=avg_s, in_=avg_s, compare_op=mybir.AluOpType.is_ge,
                fill=0.0, base=0, pattern=[[128, 4], [-K, PC]], channel_multiplier=1,
            )
            nc.gpsimd.affine_select(
                out=avg_s, in_=avg_s, compare_op=mybir.AluOpType.is_ge,
                fill=0.0, base=K - 1, pattern=[[-128, 4], [K, PC]],
                channel_multiplier=-1,
            )

            x_t = singles.tile([128, 4, D], F32, tag="x")
            nc.sync.dma_start(out=x_t, in_=x_in.rearrange("(bc p) d -> p bc d", p=128))

            # row norms: ss[b] = sum_d x[b,d]^2 ; rinv = 1/sqrt(ss)
            sq_scratch = singles.tile([128, D], F32, tag="sq")
            ss_t = singles.tile([128, 4], F32, tag="ss")
            for bc in range(4):
                nc.scalar.activation(
                    out=sq_scratch,
                    in_=x_t[:, bc],
                    func=mybir.ActivationFunctionType.Square,
                    accum_out=ss_t[:, bc : bc + 1],
                )
            std_t = singles.tile([128, 4], F32, tag="std")
            rinv_t = singles.tile([128, 4], F32, tag="rinv")
            nc.scalar.sqrt(std_t, ss_t)
            nc.vector.reciprocal(rinv_t, std_t)
            xn_t = singles.tile([128, 4, D], F32, tag="xn")
            for bc in range(4):
                nc.vector.tensor_scalar_mul(
                    xn_t[:, bc], x_t[:, bc], rinv_t[:, bc : bc + 1]
                )

            # xmT[d, pc] = sum_b xn[b, d] * A.T[b, pc]; cast to bf16 for the
            # main matmuls.
            xmT = singles.tile([128, 2, PC], BF16, tag="xmT")
            for dc in range(2):
                ps_xm = psum_pre_pool.tile([128, PC], F32, tag="ps_xm")
                for bc in range(4):
                    nc.tensor.matmul(
                        ps_xm,
                        lhsT=xn_t[:, bc, dc * 128 : (dc + 1) * 128],
                        rhs=avg_s[:, bc],
                        start=(bc == 0),
                        stop=(bc == 3),
                    )
                nc.scalar.copy(xmT[:, dc], ps_xm)

            # ---------------- main loop over feature chunks ----------------
            fT_cur = frontend(0)
            for c in range(NCHUNK):
                fT = fT_cur
                if c + 1 < NCHUNK:
                    fT_cur = frontend(c + 1)

                for s in range(CHUNK // MMN):
                    ps_mm = psum_mm_pool.tile([PC, MMN], F32, tag="ps_mm")
                    for dc in range(2):
                        rhs = fT[dc].rearrange("d j p -> d (j p)")[
                            :, s * MMN : (s + 1) * MMN
                        ]
                        nc.tensor.matmul(
                            ps_mm,
                            lhsT=xmT[:, dc],
                            rhs=rhs,
                            start=(dc == 0),
                            stop=(dc == 1),
                        )
                    # The reference drops the per-(p,c) minimum from the
                    # negative set; that term is <= Z/65536 (~1e-7 relative on
                    # the loss), so it is skipped here.
                    col = c * (CHUNK // MMN) + s
                    exp_scratch = exp_pool.tile([PC, MMN], F32, tag="exp")
                    nc.scalar.activation(
                        out=exp_scratch,
                        in_=ps_mm,
                        func=mybir.ActivationFunctionType.Exp,
                        scale=SCALE,
                        accum_out=se_cols[:, col : col + 1],
                    )

            # ---------------- epilogue: fold columns, DMA out ----------------
            se_tot = singles.tile([PC, 1], F32, tag="se_tot")
            nc.vector.tensor_reduce(
                out=se_tot, in_=se_cols, axis=mybir.AxisListType.X,
                op=mybir.AluOpType.add,
            )
            nc.sync.dma_start(out=se_out[:, :], in_=se_tot)

    nc.compile()
    return nc


_MODULE_CACHE = {}


def _get_module():
    if "nc" not in _MODULE_CACHE:
        _MODULE_CACHE["nc"] = _build_module()
    return _MODULE_CACHE["nc"]


def kernel(inputs, targets, camids, isClusterC, features, _run_kwargs=None):
    x = np.ascontiguousarray(np.asarray(inputs, dtype=np.float32))
    feats = np.ascontiguousarray(np.asarray(features, dtype=np.float32))
    assert x.shape == (B, D) and feats.shape == (N, D)

    nc = _get_module()
    in_maps = [
        {"x": x, "feat": feats[c * NSHARD : (c + 1) * NSHARD]} for c in range(NCORES)
    ]
    res = run_bass_kernel_spmd(
        nc, in_maps, core_ids=list(range(NCORES)), **(_run_kwargs or {})
    )
    results = res.results

    se = np.zeros(PC, np.float64)
    for r in results:
        se += r["sumexp"].reshape(PC).astype(np.float64)

    # own/pos: B-sized, exact in f64 on host (device dots are bf16-rounded)
    x64 = x.astype(np.float64)
    xm = (x64 / np.linalg.norm(x64, axis=1, keepdims=True)).reshape(PC, K, D).mean(1)
    own = (xm @ feats[:P].astype(np.float64).T) * SCALE           # [64, 16]

    se = se.reshape(P, C)
    pid = np.arange(P)
    own_mat = own[(pid[:, None] * C + np.arange(C)[None, :]), pid[:, None]]  # [P, C]
    pos = own_mat[pid, own_mat.argmin(1)]
    Z = np.exp(pos) + se.sum(1)
    loss = (np.log(Z) - pos).mean()
    out = np.array(loss, dtype=np.float32)
    if _run_kwargs:
        return out, res
    return out


# revision 38
# speedup vs baseline: 1.2317x; 1.2317x over previous
"""Trainium2 Bass kernel for nn_ClusterMemory (scatter_memory).

Math: the reference loss reduces exactly to
    x_mean[64,256]  = mean over K=8 of L2-normalized input rows      (B=512=16p*4c*8k)
    dots[64,N]      = x_mean @ features.T          (proxies = dots * 20)
    per (p,c) row:  sumexp = sum_n exp(20*dots),  min_exp = min_n exp(20*dots),
                    own[p,c] = 20 * dots[p*4+c, p]
    pos[p] = own[p, argmin_c own[p,:]]
    loss   = mean_p( log(exp(pos) + sum_c(sumexp - min_exp)) - pos )
(log-softmax is permutation invariant; the argsort in the reference only
drops the per-(p,c) minimum from the negative set, and exp is monotone so
the dropped term is min_exp.)

Distribution: features [65536, 256] is sharded over N across 8 cores
(8192 rows each, tensor/column parallel per the sharding hint); the
[512,256] inputs are replicated.  Each core reduces its N-shard to a
[64]-vector of sumexp and min_exp on device; the host adds 8 tiny
partials and finishes the scalar logsumexp in f64 (own/pos is B-sized,
computed on host in f64).

Per-core device pipeline (memory-bound design, ~8 MB of HBM reads):
  1. prologue: normalize x rows, fold the K-mean into a gpsimd-built
     averaging matrix, PE matmul -> xmT [256, 64] cast to bf16.
  2. per 2048-row feature chunk:
     - SWDGE (gpsimd) DMA with f32->bf16 cast: two natural-layout tiles
       natA/natB [128n, 16j*128d] (d-halves 0:128 / 128:256).
     - one SBUF->SBUF DMA-transpose each (xbar, 2-byte) -> fT [128d, 2048n].
     - PE: bf16 matmuls into PSUM [64, 512] (f32 accumulate).
     - ACT: exp(scale=20) with fused row-sum; DVE: row-min of exp.
The bf16 cast costs ~1e-4..1e-3 relative on the final scalar (PSUM
accumulation stays f32; own/pos exact on host), far inside tolerance.
"""

import numpy as np

import concourse.bass as bass
import concourse.tile as tile
from concourse import bacc, mybir
from concourse.bass_utils import run_bass_kernel_spmd

P, C, K = 16, 4, 8
B = P * C * K          # 512
N, D = 65536, 256
PC = P * C             # 64 proxy rows
NCORES = 8
NSHARD = N // NCORES   # 8192
SCALE = 20.0           # 1 / TEMP(0.05)

J = 16                 # 128-row groups per chunk
CHUNK = 128 * J        # 2048 feature rows per chunk
NCHUNK = NSHARD // CHUNK   # 4
MMN = 512              # matmul free dim / PSUM bank
F32 = mybir.dt.float32
BF16 = mybir.dt.bfloat16


def _build_module():
    nc = bacc.Bacc(
        "TRN2",
        target_bir_lowering=False,
        debug=False,
        enable_asserts=False,
        num_devices=NCORES,
    )

    x_in = nc.dram_tensor("x", [B, D], F32, kind="ExternalInput")
    f_in = nc.dram_tensor("feat", [NSHARD, D], F32, kind="ExternalInput")
    se_out = nc.dram_tensor("sumexp", [PC, 1], F32, kind="ExternalOutput")

    with tile.TileContext(nc) as tc:
        with (
            tc.tile_pool(name="singles", bufs=1) as singles,
            tc.tile_pool(name="nat", bufs=3) as nat_pool,
            tc.tile_pool(name="ftT", bufs=3) as ftT_pool,
            tc.tile_pool(name="expst", bufs=2) as exp_pool,
            tc.tile_pool(name="psum_mm", bufs=4, space="PSUM") as psum_mm_pool,
            tc.tile_pool(name="psum_pre", bufs=1, space="PSUM") as psum_pre_pool,
        ):
            # Load layout: each partition holds Q=4 *consecutive* DRAM rows so
            # DMA packets are 4KB contiguous (1KB rows alone run the SDMA
            # engines at ~69% efficiency).  Row n = c*CHUNK + g*512 + p*4 + q;
            # the resulting n-permutation is harmless (sumexp is reduced over
            # all n).
            QR = 4
            G = CHUNK // (128 * QR)   # 4 row-groups per chunk
            f_v = f_in.rearrange("(c g p q) d -> c p g q d", p=128, q=QR, g=G)
            se_cols = singles.tile([PC, NSHARD // MMN], F32, tag="se_cols")

            def frontend(c):
                """Load chunk c (f32, 4KB packets), cast to bf16 on DVE,
                xbar-transpose both d-halves SBUF->SBUF (one per HWDGE ring)."""
                nat32 = nat_pool.tile([128, G, QR, D], F32, tag="nat32", name="nat32")
                nc.sync.dma_start(out=nat32, in_=f_v[c])
                fT = [None, None]
                for dc in range(2):
                    nat = nat_pool.tile(
                        [128, G * QR, 128], BF16, tag=f"nat{dc}", name=f"nat{dc}"
                    )
                    nc.vector.tensor_copy(
                        out=nat.rearrange("p (g q) dd -> p g q dd", g=G),
                        in_=nat32[:, :, :, dc * 128 : (dc + 1) * 128],
                    )
                    fT[dc] = ftT_pool.tile(
                        [128, G * QR, 128], BF16, tag=f"fT{dc}", name=f"fT{dc}"
                    )
                    # xbar transpose SBUF->SBUF: fT[d, j, p] = nat[p, j*128+d].
                    # The ~2.4us trigger blocks the issuing sequencer, so the
                    # two d-halves go to the two HWDGE rings (SP and ACT) and
                    # the loads on SP aren't chained behind both.
                    eng = nc.sync if dc == 0 else nc.scalar
                    eng.dma_start(out=fT[dc], in_=nat, transpose=True)
                return fT

            # ---------------- prologue: xmT = xn.T @ A.T ----------------
            # avg_s[p, bc, g] = 1/8 iff g == (bc*128 + p) // 8  (row b = bc*128+p)
            avg_s = singles.tile([128, 4, PC], F32, tag="avg_s")
            nc.gpsimd.memset(avg_s, 1.0 / K)
            nc.gpsimd.affine_select(
                out=avg_s, in_=avg_s, compare_op=mybir.AluOpType.is_ge,
                fill=0.0, base=0, pattern=[[128, 4], [-K, PC]], channel_multiplier=1,
            )
            nc.gpsimd.affine_select(
                out=avg_s, in_=avg_s, compare_op=mybir.AluOpType.is_ge,
                fill=0.0, base=K - 1, pattern=[[-128, 4], [K, PC]],
                channel_multiplier=-1,
            )

            x_t = singles.tile([128, 4, D], F32, tag="x")
            nc.sync.dma_start(out=x_t, in_=x_in.rearrange("(bc p) d -> p bc d", p=128))

            # row norms: ss[b] = sum_d x[b,d]^2 ; rinv = 1/sqrt(ss)
            sq_scratch = singles.tile([128, D], F32, tag="sq")
            ss_t = singles.tile([128, 4], F32, tag="ss")
            for bc in range(4):
                nc.scalar.activation(
                    out=sq_scratch,
                    in_=x_t[:, bc],
                    func=mybir.ActivationFunctionType.Square,
                    accum_out=ss_t[:, bc : bc + 1],
                )
            std_t = singles.tile([128, 4], F32, tag="std")
            rinv_t = singles.tile([128, 4], F32, tag="rinv")
            nc.scalar.sqrt(std_t, ss_t)
            nc.vector.reciprocal(rinv_t, std_t)
            xn_t = singles.tile([128, 4, D], F32, tag="xn")
            for bc in range(4):
                nc.vector.tensor_scalar_mul(
                    xn_t[:, bc], x_t[:, bc], rinv_t[:, bc : bc + 1]
                )

            # xmT[d, pc] = sum_b xn[b, d] * A.T[b, pc]; cast to bf16 for the
            # main matmuls.
            xmT = singles.tile([128, 2, PC], BF16, tag="xmT")
            for dc in range(2):
                ps_xm = psum_pre_pool.tile([128, PC], F32, tag="ps_xm")
                for bc in range(4):
                    nc.tensor.matmul(
                        ps_xm,
                        lhsT=xn_t[:, bc, dc * 128 : (dc + 1) * 128],
                        rhs=avg_s[:, bc],
                        start=(bc == 0),
                        stop=(bc == 3),
                    )
                nc.scalar.copy(xmT[:, dc], ps_xm)

            # ---------------- main loop over feature chunks ----------------
            fT_cur = frontend(0)
            for c in range(NCHUNK):
                fT = fT_cur
                if c + 1 < NCHUNK:
                    fT_cur = frontend(c + 1)

                for s in range(CHUNK // MMN):
                    ps_mm = psum_mm_pool.tile([PC, MMN], F32, tag="ps_mm")
                    for dc in range(2):
                        rhs = fT[dc].rearrange("d j p -> d (j p)")[
                            :, s * MMN : (s + 1) * MMN
                        ]
                        nc.tensor.matmul(
                            ps_mm,
                            lhsT=xmT[:, dc],
                            rhs=rhs,
                            start=(dc == 0),
                            stop=(dc == 1),
                        )
                    # The reference drops the per-(p,c) minimum from the
                    # negative set; that term is <= Z/65536 (~1e-7 relative on
                    # the loss), so it is skipped here.
                    col = c * (CHUNK // MMN) + s
                    exp_scratch = exp_pool.tile([PC, MMN], F32, tag="exp")
                    nc.scalar.activation(
                        out=exp_scratch,
                        in_=ps_mm,
                        func=mybir.ActivationFunctionType.Exp,
                        scale=SCALE,
                        accum_out=se_cols[:, col : col + 1],
                    )

            # ---------------- epilogue: fold columns, DMA out ----------------
            se_tot = singles.tile([PC, 1], F32, tag="se_tot")
            nc.vector.tensor_reduce(
                out=se_tot, in_=se_cols, axis=mybir.AxisListType.X,
                op=mybir.AluOpType.add,
            )
            nc.sync.dma_start(out=se_out[:, :], in_=se_tot)

    nc.compile()
    return nc


_MODULE_CACHE = {}


def _get_module():
    if "nc" not in _MODULE_CACHE:
        _MODULE_CACHE["nc"] = _build_module()
    return _MODULE_CACHE["nc"]


def kernel(inputs, targets, camids, isClusterC, features, _run_kwargs=None):
    x = np.ascontiguousarray(np.asarray(inputs, dtype=np.float32))
    feats = np.ascontiguousarray(np.asarray(features, dtype=np.float32))
    assert x.shape == (B, D) and feats.shape == (N, D)

    nc = _get_module()
    in_maps = [
        {"x": x, "feat": feats[c * NSHARD : (c + 1) * NSHARD]} for c in range(NCORES)
    ]
    res = run_bass_kernel_spmd(
        nc, in_maps, core_ids=list(range(NCORES)), **(_run_kwargs or {})
    )
    results = res.results

    se = np.zeros(PC, np.float64)
    for r in results:
        se += r["sumexp"].reshape(PC).astype(np.float64)

    # own/pos: B-sized, exact in f64 on host (device dots are bf16-rounded)
    x64 = x.astype(np.float64)
    xm = (x64 / np.linalg.norm(x64, axis=1, keepdims=True)).reshape(PC, K, D).mean(1)
    own = (xm @ feats[:P].astype(np.float64).T) * SCALE           # [64, 16]

    se = se.reshape(P, C)
    pid = np.arange(P)
    own_mat = own[(pid[:, None] * C + np.arange(C)[None, :]), pid[:, None]]  # [P, C]
    pos = own_mat[pid, own_mat.argmin(1)]
    Z = np.exp(pos) + se.sum(1)
    loss = (np.log(Z) - pos).mean()
    out = np.array(loss, dtype=np.float32)
    if _run_kwargs:
        return out, res
    return out
